# revision 1
# baseline (speedup 1.0000x reference)
"""Trainium2 Bass kernel for 2-layer GCN (N=50000, E=600000, 128->512->128).

Strategy (8 NeuronCores, graph/data parallel over destination nodes):
  - Host: symmetric-normalization is separable (norm = dinv[src]*dinv[dst]),
    so the gather table rows are pre-scaled by dinv[src] and the aggregate is
    post-scaled by dinv[dst]. Nodes are packed into 8*49 windows of <=128
    destination nodes, balancing per-window edge counts so one SPMD program
    (fixed shapes) serves all cores. Edge slots are split into two source
    ranges (A: table rows [0,32768), B: rows [17234,50002)) so gather indices
    fit int16, padded per window to NA*128 / NB*128 slots.
  - Device, per window: dma_gather fp16 source rows (256B rows, two windows
    per gather, single_packet=False) -> build all of a window-range's one-hot
    S matrices with ONE wide DVE is_equal against a broadcast iota (batched to
    amortize per-op overhead) -> PE matmuls accumulate the aggregation in
    PSUM (operands swapped per layer so each consumer gets its layout without
    transposes). PSUM drains ride the otherwise-idle ACT engine. Layer 1
    continues on-chip: agg -> @W1^T -> +b1,relu -> @W2^T -> z (so only the
    128-wide z crosses HBM between layers). Layer 2 finishes:
    agg*dinv[dst] + b2 -> relu -> out.
  - Host between launches: reshuffles z shards into the layer-2 gather table
    (scaled by dinv), then un-permutes the final output.
"""

import heapq
import numpy as np

import concourse.bacc as bacc
import concourse.mybir as mybir
import concourse.tile as tile
from concourse.bass_utils import run_bass_kernel_spmd

# problem constants (hardcoded per contract)
N = 50000
E = 600000
F = 128          # in/out feature dim
H = 512          # hidden dim
P = 128
NCORES = 8
WPC = 49                  # windows per core
BINS = NCORES * WPC       # 392
ROWS_PER_CORE = WPC * P   # 6272 output rows per core (>= 6250 real)
TBL_ROWS = N + 2          # zero row at 0 and N+1
A_MAX_SRC = 32766         # srcs <= this go to range A (idx = src+1 <= 32767)
B_OFF = 17234             # range B table view starts at this row
B_PAD_IDX = 32767         # row N+1 (zero) relative to B view
SENTINEL = 300.0          # dstloc value that never matches iota 0..127
GW = 2                    # windows per gather group

last_run_info = {}


# ---------------------------------------------------------------- host planner
def _pack_bins(a_tot, b_tot, cap_a, cap_b):
    """Greedy balanced packing of nodes into BINS bins (<=P nodes, slot caps).
    Returns per-node bin id, or None if packing failed."""
    order = np.argsort(-(a_tot * 3 + b_tot))  # heaviest first
    bin_of = np.full(N, -1, np.int32)
    heap = [(0, 0, 0, b) for b in range(BINS)]  # (aload, bload, count, bin)
    heapq.heapify(heap)
    for n in order:
        a, b = int(a_tot[n]), int(b_tot[n])
        tried = []
        placed = False
        while heap:
            al, bl, cnt, bid = heapq.heappop(heap)
            if cnt >= P:
                continue  # bin full: drop permanently
            if al + a <= cap_a and bl + b <= cap_b:
                bin_of[n] = bid
                heapq.heappush(heap, (al + a, bl + b, cnt + 1, bid))
                placed = True
                break
            tried.append((al, bl, cnt, bid))
            if len(tried) > 256:
                break
        for t in tried:
            heapq.heappush(heap, t)
        if not placed:
            return None
    return bin_of


def build_plan(edge_index):
    src = np.asarray(edge_index[0], dtype=np.int64).astype(np.int32)
    dst = np.asarray(edge_index[1], dtype=np.int64).astype(np.int32)

    deg = np.bincount(dst, minlength=N).astype(np.int64) + 1  # + self loop
    dinv = (1.0 / np.sqrt(deg)).astype(np.float32)

    is_a = src <= A_MAX_SRC
    a_cnt = np.bincount(dst[is_a], minlength=N)
    b_cnt = np.bincount(dst[~is_a], minlength=N)
    self_a = np.arange(N) <= A_MAX_SRC
    a_tot = a_cnt + self_a
    b_tot = b_cnt + (~self_a)

    for na, nb in ((9, 5), (9, 6), (10, 6), (10, 8), (12, 10)):
        bin_of = _pack_bins(a_tot, b_tot, na * P, nb * P)
        if bin_of is not None:
            NA, NB = na, nb
            break
    else:
        raise RuntimeError("bin packing failed")

    # per-bin node lists / positions
    node_core = bin_of // WPC
    node_win = bin_of % WPC
    node_pos = np.zeros(N, np.int32)
    fill = np.zeros(BINS, np.int32)
    for n in range(N):
        b = bin_of[n]
        node_pos[n] = fill[b]
        fill[b] += 1

    # CSR of incoming edges per node (edges only; self loop added below)
    order = np.argsort(dst, kind="stable")
    src_sorted = src[order]
    starts = np.zeros(N + 1, np.int64)
    np.cumsum(np.bincount(dst, minlength=N), out=starts[1:])

    slots_a = NA * P
    slots_b = NB * P
    idxA = np.zeros((NCORES, WPC, slots_a), np.int16)
    idxB = np.full((NCORES, WPC, slots_b), B_PAD_IDX, np.int16)
    dlA = np.full((NCORES, WPC, slots_a), SENTINEL, np.float32)
    dlB = np.full((NCORES, WPC, slots_b), SENTINEL, np.float32)
    fa = np.zeros((NCORES, WPC), np.int32)
    fb = np.zeros((NCORES, WPC), np.int32)
    dinvw = np.zeros((NCORES, WPC, P), np.float32)

    for n in range(N):
        c, w, p = node_core[n], node_win[n], node_pos[n]
        dinvw[c, w, p] = dinv[n]
        es = src_sorted[starts[n]:starts[n + 1]]
        ea = es[es <= A_MAX_SRC]
        eb = es[es > A_MAX_SRC]
        if n <= A_MAX_SRC:
            ea = np.append(ea, n)
        else:
            eb = np.append(eb, n)
        ka, kb = len(ea), len(eb)
        oa, ob = fa[c, w], fb[c, w]
        idxA[c, w, oa:oa + ka] = (ea + 1).astype(np.int16)
        dlA[c, w, oa:oa + ka] = p
        idxB[c, w, ob:ob + kb] = (eb - (B_OFF - 1)).astype(np.int16)
        dlB[c, w, ob:ob + kb] = p
        fa[c, w] += ka
        fb[c, w] += kb

    # device layouts
    def wrap_idx(arr, ns):  # [NCORES, WPC, ns] -> [NCORES, 128, WPC*ns//16]
        a = arr.reshape(NCORES, WPC, ns // 16, 16)
        a = np.transpose(a, (0, 3, 1, 2)).reshape(NCORES, 16, WPC * (ns // 16))
        return np.tile(a, (1, 8, 1)).copy()

    def wrap_dl(arr, ns):  # -> [NCORES, 128, WPC*(ns//128)]
        a = arr.reshape(NCORES, WPC, ns // P, P)
        return np.transpose(a, (0, 3, 1, 2)).reshape(NCORES, P, WPC * (ns // P)).copy()

    plan = dict(
        NA=NA, NB=NB, dinv=dinv,
        idxA=wrap_idx(idxA, slots_a), idxB=wrap_idx(idxB, slots_b),
        dlA=wrap_dl(dlA, slots_a), dlB=wrap_dl(dlB, slots_b),
        dinvw=dinvw,                                    # [NCORES, WPC, P]
        dinvp=np.transpose(dinvw, (0, 2, 1)).copy(),    # [NCORES, P, WPC]
        node_core=node_core, node_row=node_win * P + node_pos,
    )
    return plan


def make_table(feat, dinv):
    """[TBL_ROWS, F] f16 table: row n+1 = dinv[n] * feat[n]; rows 0, N+1 zero."""
    t = np.zeros((TBL_ROWS, F), np.float16)
    t[1:N + 1] = (feat * dinv[:, None]).astype(np.float16)
    return t


# ---------------------------------------------------------------- device kernel
def build_kernel(layer, NA, NB, wpc=WPC, use_b=True, s16=False,
                 msg_bufs=2, s_bufs=4, wk_bufs=2, ps_bufs=None, act_drain=False,
                 probe=None):
    """layer 1: table -> z = relu(agg*dinvw @ W1T + b1) @ W2T   (out [6272,128])
    layer 2: table -> out = relu(agg*dinvp + b2)               (out [6272,128])
    """
    f32, f16, i16 = mybir.dt.float32, mybir.dt.float16, mybir.dt.int16
    fdl = f16 if s16 else f32
    if ps_bufs is None:
        ps_bufs = 2 if layer == 1 else 4
    nc = bacc.Bacc("TRN2", debug=False)
    d = {}
    d["table"] = nc.dram_tensor("table", [TBL_ROWS, F], f16, kind="ExternalInput").ap()
    d["idxA"] = nc.dram_tensor("idxA", [P, wpc * NA * 8], i16, kind="ExternalInput").ap()
    d["idxB"] = nc.dram_tensor("idxB", [P, wpc * NB * 8], i16, kind="ExternalInput").ap()
    d["dlA"] = nc.dram_tensor("dlA", [P, wpc * NA], fdl, kind="ExternalInput").ap()
    d["dlB"] = nc.dram_tensor("dlB", [P, wpc * NB], fdl, kind="ExternalInput").ap()
    d["iota"] = nc.dram_tensor("iota", [P, P], fdl, kind="ExternalInput").ap()
    if layer == 1:
        d["dinvw"] = nc.dram_tensor("dinvw", [P, wpc * P], f32, kind="ExternalInput").ap()
        d["w1t"] = nc.dram_tensor("w1t", [P, H], f16, kind="ExternalInput").ap()
        d["b1c"] = nc.dram_tensor("b1c", [P, H // P], f32, kind="ExternalInput").ap()
        d["w2t"] = nc.dram_tensor("w2t", [P, H], f16, kind="ExternalInput").ap()
    else:
        d["dinvp"] = nc.dram_tensor("dinvp", [P, wpc], f32, kind="ExternalInput").ap()
        d["b2r"] = nc.dram_tensor("b2r", [P, P], f32, kind="ExternalInput").ap()
    out_d = nc.dram_tensor("out", [wpc * P, F], f32, kind="ExternalOutput").ap()

    Relu = mybir.ActivationFunctionType.Relu
    Copy = mybir.ActivationFunctionType.Copy

    WPCl = wpc
    with tile.TileContext(nc) as tc:
        with (
            tc.tile_pool(name="cst", bufs=1) as cp,
            tc.tile_pool(name="msg", bufs=msg_bufs) as mp,
            tc.tile_pool(name="s", bufs=s_bufs) as spool,
            tc.tile_pool(name="work", bufs=wk_bufs) as wp,
            tc.tile_pool(name="psum", bufs=ps_bufs, space="PSUM") as pp,
        ):
            def load(name, shape, dtype):
                t = cp.tile(shape, dtype, tag=name)
                nc.sync.dma_start(out=t[:], in_=d[name][:])
                return t

            idxA_t = load("idxA", [P, wpc * NA * 8], i16)
            idxB_t = load("idxB", [P, wpc * NB * 8], i16)
            dlA_t = load("dlA", [P, wpc * NA], fdl)
            dlB_t = load("dlB", [P, wpc * NB], fdl)
            iota_t = load("iota", [P, P], fdl)
            if layer == 1:
                dinvw_t = load("dinvw", [P, wpc * P], f32)
                w1t_t = load("w1t", [P, H], f16)
                b1c_t = load("b1c", [P, H // P], f32)
                w2t_t = load("w2t", [P, H], f16)
            else:
                dinvp_t = load("dinvp", [P, wpc], f32)
                b2r_t = load("b2r", [P, P], f32)

            sconst = None
            if probe == "noS":
                sconst = cp.tile([P, P], f16, tag="sconst")
                nc.vector.tensor_tensor(out=sconst[:],
                                        in0=dlA_t[:, 0:1].to_broadcast([P, P]),
                                        in1=iota_t[:], op=mybir.AluOpType.is_equal)
            for g0 in range(0, wpc, GW):
                nw = min(GW, wpc - g0)
                ja, jb = nw * NA, nw * NB
                msgs16 = {}
                ranges = [("A", NA, dlA_t)] + ([("B", NB, dlB_t)] if use_b else [])
                for rng, nj, idx_t, npc in ([("A", ja, idxA_t, NA)] + ([("B", jb, idxB_t, NB)] if use_b else [])):
                    mt = mp.tile([P, nj * F], f16, tag=f"m{rng}")
                    in_ap = d["table"][:] if rng == "A" else d["table"][B_OFF:TBL_ROWS, :]
                    nc.gpsimd.dma_gather(
                        out_ap=mt[:].rearrange("p (j e) -> p j e", e=F),
                        in_ap=in_ap,
                        idxs_ap=idx_t[:, g0 * npc * 8:(g0 * npc + nj) * 8],
                        num_idxs=(P if probe == "smallG" else nj * P),
                        num_idxs_reg=(P if probe == "smallG" else nj * P),
                        elem_size=F,
                        single_packet=False,
                    )
                    msgs16[rng] = mt

                for wi in range(nw):
                    w = g0 + wi
                    agg = pp.tile([P, P], f32, tag="agg")
                    nmm = NA + (NB if use_b else 0)
                    k = 0
                    for rng, npc, dl_t in ranges:
                        if probe != "noS":
                            sw_t = spool.tile([P, npc * P], f16, tag=f"s{rng}")
                            nc.vector.tensor_tensor(
                                out=sw_t[:].rearrange("p (c e) -> p c e", e=P),
                                in0=dl_t[:, w * npc:(w + 1) * npc]
                                    .unsqueeze(-1).to_broadcast([P, npc, P]),
                                in1=iota_t[:].unsqueeze(1).to_broadcast([P, npc, P]),
                                op=mybir.AluOpType.is_equal,
                            )
                        for c in range(npc):
                            if probe == "noS":
                                s_t = sconst[:]
                            else:
                                s_t = sw_t[:, c * P:(c + 1) * P]
                            mm = msgs16[rng][:, ((wi * npc + c) if probe != "smallG" else 0) * F:((wi * npc + c) if probe != "smallG" else 0) * F + F]
                            if probe == "noPE" and not (k == 0 or k == nmm - 1):
                                k += 1
                                continue
                            if layer == 1:
                                # aggT[f, d] += msg^T @ S
                                nc.tensor.matmul(out=agg[:], lhsT=mm, rhs=s_t,
                                                 start=(k == 0), stop=(k == nmm - 1))
                            else:
                                # agg[d, f] += S^T @ msg
                                nc.tensor.matmul(out=agg[:], lhsT=s_t, rhs=mm,
                                                 start=(k == 0), stop=(k == nmm - 1))
                            k += 1

                    if layer == 1:
                        # aggTs[f, d] = aggT * dinv[dst]  (free-dim broadcast row)
                        aggs = wp.tile([P, P], f16, tag="aggs")
                        dr = dinvw_t[:, w * P:(w + 1) * P]
                        nc.vector.tensor_tensor(out=aggs[:], in0=agg[:], in1=dr,
                                                op=mybir.AluOpType.mult)
                        hts = []
                        hps = pp.tile([P, H], f32, tag="h")
                        for oc in range(H // P):
                            nc.tensor.matmul(
                                out=hps[:, oc * P:(oc + 1) * P],
                                lhsT=w1t_t[:, oc * P:(oc + 1) * P],
                                rhs=aggs[:], start=True, stop=True)
                            ht = wp.tile([P, P], f16, tag=f"ht{oc}")
                            nc.scalar.activation(out=ht[:], in_=hps[:, oc * P:(oc + 1) * P],
                                                 func=Relu, bias=b1c_t[:, oc:oc + 1])
                            hts.append(ht)
                        zps = pp.tile([P, P], f32, tag="z")
                        for ic in range(H // P):
                            nc.tensor.matmul(out=zps[:], lhsT=hts[ic][:],
                                             rhs=w2t_t[:, ic * P:(ic + 1) * P],
                                             start=(ic == 0), stop=(ic == H // P - 1))
                        zsb = wp.tile([P, P], f32, tag="zsb")
                        if act_drain:
                            nc.scalar.activation(out=zsb[:], in_=zps[:], func=Copy)
                        else:
                            nc.vector.tensor_copy(out=zsb[:], in_=zps[:])
                        nc.sync.dma_start(out=out_d[w * P:(w + 1) * P, :], in_=zsb[:])
                    else:
                        u = wp.tile([P, P], f32, tag="u")
                        if act_drain:
                            nc.scalar.activation(out=u[:], in_=agg[:], func=Copy,
                                                 scale=dinvp_t[:, w:w + 1])
                        else:
                            nc.vector.tensor_scalar_mul(u[:], agg[:], dinvp_t[:, w:w + 1])
                        v = wp.tile([P, P], f32, tag="v")
                        nc.vector.tensor_tensor(out=v[:], in0=u[:],
                                                in1=b2r_t[:],
                                                op=mybir.AluOpType.add)
                        y = wp.tile([P, P], f32, tag="y")
                        nc.scalar.activation(out=y[:], in_=v[:], func=Relu)
                        nc.sync.dma_start(out=out_d[w * P:(w + 1) * P, :], in_=y[:])

    nc.compile()
    return nc


# ---------------------------------------------------------------- entry point
def _in_maps(plan, layer, table, W1=None, b1=None, W2=None, b2=None):
    iota = np.broadcast_to(np.arange(P, dtype=np.float32), (P, P)).copy()
    maps = []
    for c in range(NCORES):
        m = dict(table=table, iota=iota,
                 idxA=plan["idxA"][c], idxB=plan["idxB"][c],
                 dlA=plan["dlA"][c], dlB=plan["dlB"][c])
        if layer == 1:
            m["dinvw"] = np.broadcast_to(
                plan["dinvw"][c].reshape(1, WPC * P), (P, WPC * P)).copy()
            m["w1t"] = W1.T.astype(np.float16).copy()
            m["b1c"] = b1.reshape(H // P, P).T.astype(np.float32).copy()
            m["w2t"] = np.concatenate(
                [W2[:, c0 * P:(c0 + 1) * P].T for c0 in range(H // P)], axis=1
            ).astype(np.float16).copy()
        else:
            m["dinvp"] = plan["dinvp"][c]
            m["b2r"] = np.broadcast_to(b2.astype(np.float32), (P, P)).copy()
        maps.append(m)
    return maps


def _gather_nodes(plan, outs):
    """[NCORES][ROWS_PER_CORE, F] core outputs -> [N, F] in node order."""
    allo = np.stack(outs)  # [NCORES, ROWS_PER_CORE, F]
    return allo[plan["node_core"], plan["node_row"]]


def kernel(**inputs):
    x = np.asarray(inputs["x"], np.float32)
    edge_index = np.asarray(inputs["edge_index"])
    W1 = np.asarray(inputs["W1"], np.float32)
    b1 = np.asarray(inputs["b1"], np.float32)
    W2 = np.asarray(inputs["W2"], np.float32)
    b2 = np.asarray(inputs["b2"], np.float32)

    plan = build_plan(edge_index)
    nc1 = build_kernel(1, plan["NA"], plan["NB"], act_drain=True, wk_bufs=3)
    nc2 = build_kernel(2, plan["NA"], plan["NB"], act_drain=True, wk_bufs=3)

    t1 = make_table(x, plan["dinv"])
    r1 = run_bass_kernel_spmd(
        nc1, _in_maps(plan, 1, t1, W1=W1, b1=b1, W2=W2), core_ids=list(range(NCORES)))
    z = _gather_nodes(plan, [r1.results[c]["out"] for c in range(NCORES)])

    t2 = make_table(z, plan["dinv"])
    r2 = run_bass_kernel_spmd(
        nc2, _in_maps(plan, 2, t2, b2=b2), core_ids=list(range(NCORES)))
    y = _gather_nodes(plan, [r2.results[c]["out"] for c in range(NCORES)])

    last_run_info["exec_time_ns"] = [r1.exec_time_ns, r2.exec_time_ns]
    last_run_info["ncs"] = (nc1, nc2)
    return y.astype(np.float32)



# revision 23
# speedup vs baseline: 1.6492x; 1.6492x over previous
"""Trainium2 Bass kernel for 2-layer GCN (N=50000, E=600000, 128->512->128).

Strategy (8 NeuronCores, graph/data parallel over destination nodes):
  - Aggregate-then-transform: segment_sum commutes with the linear layers, so
    both layers aggregate 128-wide features.  Symmetric normalization is
    separable: table rows are pre-scaled by dinv[src], aggregates post-scaled
    by dinv[dst].
  - Identity-S packing: nodes are greedily packed into 392 windows of 128
    destination nodes minimizing sum(maxA+maxB) chunk counts; window chunk c
    holds, at slot p, the c-th incoming edge of the window's p-th node, so the
    PE aggregates each chunk against a constant identity matrix (no per-window
    one-hot build).  Windows are grouped 8-to-a-position across cores (one
    SPMD shape) and 4-positions-to-a-group so each chunk matmul carries a
    512-wide moving operand: PE sequencer issue rate, not FLOPs, is the
    limiter.
  - fp8(e3m4) gather table, rows on a 256B stride: dma_gather is emitted
    directly (the 256B elem_size assert is a transpose-only Q7 restriction;
    the stride must be 256B-aligned and is), so each edge moves 128 bytes.
    Tables are optimally scaled per layer on the host; inverse scales fold
    into the dinv normalization multipliers.
  - Layer 1 on-chip: agg4[d,(wi,f)] -> transpose -> @W1^T, relu -> @W2^T ->
    zT*(dinv^2/s1) -> fp16 (b1==0 lets dinv commute past relu; a general b1
    path applies dinv/s1 before the biased relu).  Host rescales z into the
    layer-2 table.  Layer 2: agg4 -> relu(agg*dinv/s2 + b2) -> y fp16.
"""

import numpy as np

import concourse.bacc as bacc
import concourse.mybir as mybir
import concourse.tile as tile
from concourse.bass_utils import run_bass_kernel_spmd

# problem constants (hardcoded per contract)
N = 50000
E = 600000
F = 128          # in/out feature dim
H = 512          # hidden dim
P = 128
NCORES = 8
WPC = 49                  # window positions per core
BINS = NCORES * WPC       # 392
ROWS_PER_CORE = WPC * P   # 6272 output rows per core (>= 6250 real)
TBL_ROWS = N + 2          # zero row at 0 and N+1
TBL_STRIDE = 256          # fp8 elems between rows (256B stride, 128B payload)
A_MAX_SRC = 32766         # srcs <= this go to range A (idx = src+1 <= 32767)
B_OFF = 17234             # range B table view starts at this row
B_PAD_IDX = 32767         # row N+1 (zero) relative to B view
NG = 4                    # positions per matmul group (512-wide moving ops)

F8 = mybir.dt.float8e3
F8NP = mybir.dt.np(F8)    # ml_dtypes.float8_e3m4
F8MAX = 15.5

GROUPS = [(g0, min(NG, WPC - g0)) for g0 in range(0, WPC, NG)]  # (start, width)

last_run_info = {}


# ---------------------------------------------------------------- host planner
def build_plan(edge_index):
    src = np.asarray(edge_index[0]).astype(np.int64)
    dst = np.asarray(edge_index[1]).astype(np.int64)

    deg = np.bincount(dst, minlength=N).astype(np.int64) + 1  # + self loop
    dinv = (1.0 / np.sqrt(deg)).astype(np.float32)

    iota = np.arange(N, dtype=np.int64)
    e_isa = src <= A_MAX_SRC
    a_cnt = np.bincount(dst[e_isa], minlength=N) + (iota <= A_MAX_SRC)
    b_cnt = np.bincount(dst[~e_isa], minlength=N) + (iota > A_MAX_SRC)

    # stage 1: greedy-pack nodes into 392 windows minimizing sum(maxA+maxB)
    order = np.lexsort((-b_cnt, -(a_cnt + b_cnt)))  # heaviest first
    ka = np.zeros(BINS)
    kb = np.zeros(BINS)
    cnt = np.zeros(BINS)
    binof = np.zeros(N, np.int64)
    for n in order:
        an, bn = a_cnt[n], b_cnt[n]
        pen = np.maximum(an - ka, 0) + np.maximum(bn - kb, 0)
        pen[cnt >= P] = 1e9
        j = int(np.argmin(pen * 100000 - cnt))
        binof[n] = j
        ka[j] = max(ka[j], an)
        kb[j] = max(kb[j], bn)
        cnt[j] += 1

    # stage 2: group the 392 windows into 49 SPMD positions of 8 cores
    worder = np.lexsort((-kb, -ka))
    gof = np.zeros(BINS, np.int64)
    gof[worder] = np.arange(BINS) // NCORES

    def group_cost(g):
        gka = np.zeros(WPC)
        gkb = np.zeros(WPC)
        np.maximum.at(gka, g, ka)
        np.maximum.at(gkb, g, kb)
        return np.maximum(gka, 1).sum() + np.maximum(gkb, 1).sum()

    rng = np.random.default_rng(0)
    cur = group_cost(gof)
    for _ in range(30000):
        i, j = rng.integers(0, BINS, 2)
        if gof[i] == gof[j]:
            continue
        gof[i], gof[j] = gof[j], gof[i]
        c2 = group_cost(gof)
        if c2 < cur:
            cur = c2
        else:
            gof[i], gof[j] = gof[j], gof[i]

    gka = np.zeros(WPC)
    gkb = np.zeros(WPC)
    np.maximum.at(gka, gof, ka)
    np.maximum.at(gkb, gof, kb)
    kA = np.maximum(gka, 1).astype(np.int64)
    kB = np.maximum(gkb, 1).astype(np.int64)

    # re-sort positions by profile, then DP-partition into groups of width<=NG
    # (variable width trades gather padding against PE instruction count),
    # then swap-refine memberships
    porder = np.lexsort((-kB, -kA))
    a_s, b_s = kA[porder], kB[porder]
    LAM = 200.0
    INF = float("inf")
    best = [INF] * (WPC + 1)
    best[0] = 0.0
    chw = [0] * (WPC + 1)
    for i in range(1, WPC + 1):
        for w in range(1, NG + 1):
            if i - w < 0:
                break
            c = best[i - w] + (w * P + LAM) * (a_s[i - w:i].max()
                                               + b_s[i - w:i].max())
            if c < best[i]:
                best[i] = c
                chw[i] = w
    memb = []
    i = WPC
    while i > 0:
        w = chw[i]
        memb.append(list(range(i - w, i)))
        i -= w
    memb.reverse()

    def gcost(m):
        return len(m) * (a_s[m].max() + b_s[m].max())

    gcosts = [gcost(np.array(m)) for m in memb]
    for _ in range(20000):
        gi, gj = rng.integers(0, len(memb), 2)
        if gi == gj:
            continue
        mi, mj = memb[gi], memb[gj]
        ii, jj = rng.integers(0, len(mi)), rng.integers(0, len(mj))
        mi[ii], mj[jj] = mj[jj], mi[ii]
        c1, c2 = gcost(np.array(mi)), gcost(np.array(mj))
        if c1 + c2 < gcosts[gi] + gcosts[gj]:
            gcosts[gi], gcosts[gj] = c1, c2
        else:
            mi[ii], mj[jj] = mj[jj], mi[ii]

    # relabel positions so each group's positions are consecutive,
    # each group internally sorted by (kA desc, kB desc) so chunk widths
    # are (near-)prefixes
    for m in memb:
        m.sort(key=lambda si: (-a_s[si], -b_s[si]))
    flat = [s for m in memb for s in m]       # sorted-space index -> order
    newpos_of_sorted = np.empty(WPC, np.int64)
    newpos_of_sorted[flat] = np.arange(WPC)
    pos_new = np.empty(WPC, np.int64)
    pos_new[porder] = newpos_of_sorted        # original pos -> new pos
    gof = pos_new[gof]
    kA2 = np.empty(WPC, np.int64)
    kB2 = np.empty(WPC, np.int64)
    kA2[newpos_of_sorted] = a_s
    kB2[newpos_of_sorted] = b_s
    kA, kB = kA2, kB2
    groups = []
    off = 0
    for m in memb:
        groups.append((off, len(m)))
        off += len(m)

    # per-group variable-width chunks: chunk c spans the first mA_c windows
    # (A side, prefix by sort) / up to the last window with kB>c (B side)
    kAg = np.array([int(kA[g0:g0 + nw].max()) for g0, nw in groups])
    kBg = np.array([int(kB[g0:g0 + nw].max()) for g0, nw in groups])
    chwA, choffA, chwB, choffB = [], [], [], []
    for gi, (g0, nw) in enumerate(groups):
        wa = [int(np.sum(kA[g0:g0 + nw] > c)) for c in range(kAg[gi])]
        wb = [int(np.max(np.nonzero(kB[g0:g0 + nw] > c)[0])) + 1
              for c in range(kBg[gi])]
        chwA.append(wa)
        chwB.append(wb)
        choffA.append(np.concatenate([[0], np.cumsum(wa)]).astype(np.int64) * P)
        choffB.append(np.concatenate([[0], np.cumsum(wb)]).astype(np.int64) * P)
    soffA = np.concatenate([[0], np.cumsum([o[-1] for o in choffA])]).astype(
        np.int64)
    soffB = np.concatenate([[0], np.cumsum([o[-1] for o in choffB])]).astype(
        np.int64)
    SA, SB = int(soffA[-1]), int(soffB[-1])

    # node -> (core, position, slot)
    corein = np.zeros(BINS, np.int64)
    for w in range(WPC):
        bw = np.where(gof == w)[0]
        corein[bw] = np.arange(len(bw))
    node_core = corein[binof]
    node_w = gof[binof]
    bsort = np.argsort(binof, kind="stable")
    bcounts = np.bincount(binof, minlength=BINS)
    bstarts = np.concatenate([[0], np.cumsum(bcounts)[:-1]])
    node_p = np.empty(N, np.int64)
    node_p[bsort] = np.arange(N) - bstarts[binof[bsort]]
    node_row = node_w * P + node_p

    # edges + self loops, ranked within (dst node, range class)
    es = np.concatenate([src, iota])
    ed = np.concatenate([dst, iota])
    ecls = (es > A_MAX_SRC).astype(np.int64)
    key = ed * 2 + ecls
    eorder = np.argsort(key, kind="stable")
    ks = key[eorder]
    counts = np.bincount(key, minlength=2 * N)
    starts = np.concatenate([[0], np.cumsum(counts)[:-1]])
    rank = np.arange(len(es)) - starts[ks]
    se, de = es[eorder], ed[eorder]

    pos2g = np.empty(WPC, np.int64)
    pos2wi = np.empty(WPC, np.int64)
    for gi, (g0, nw) in enumerate(groups):
        pos2g[g0:g0 + nw] = gi
        pos2wi[g0:g0 + nw] = np.arange(nw)
    c_, w_, p_ = node_core[de], node_w[de], node_p[de]
    g_, wi_ = pos2g[w_], pos2wi[w_]
    flatoffA = np.zeros((len(groups), int(kAg.max()) + 1), np.int64)
    flatoffB = np.zeros((len(groups), int(kBg.max()) + 1), np.int64)
    for gi in range(len(groups)):
        flatoffA[gi, :len(choffA[gi])] = soffA[gi] + choffA[gi]
        flatoffB[gi, :len(choffB[gi])] = soffB[gi] + choffB[gi]
    mA = ks % 2 == 0
    idxA_flat = np.zeros(NCORES * SA, np.int16)
    linA = (c_[mA] * SA + flatoffA[g_[mA], rank[mA]] + wi_[mA] * P + p_[mA])
    idxA_flat[linA] = (se[mA] + 1).astype(np.int16)
    mB = ~mA
    idxB_flat = np.full(NCORES * SB, B_PAD_IDX, np.int16)
    linB = (c_[mB] * SB + flatoffB[g_[mB], rank[mB]] + wi_[mB] * P + p_[mB])
    idxB_flat[linB] = (se[mB] + 1 - B_OFF).astype(np.int16)

    def wrap(flat, stot):  # [NCORES, stot] -> [NCORES, 128, stot/16]
        a = flat.reshape(NCORES, stot // 16, 16)
        a = np.transpose(a, (0, 2, 1))
        return np.tile(a, (1, 8, 1)).copy()

    dinvrow = np.zeros((NCORES, ROWS_PER_CORE), np.float32)
    dinvrow[node_core, node_row] = dinv

    return dict(
        kA=kA, kB=kB, kAg=kAg, kBg=kBg, soffA=soffA, soffB=soffB, dinv=dinv,
        chwA=chwA, chwB=chwB, choffA=choffA, choffB=choffB,
        groups=groups, idxA=wrap(idxA_flat, SA), idxB=wrap(idxB_flat, SB),
        dinvrow=dinvrow, node_core=node_core, node_row=node_row,
        node_w=node_w, node_p=node_p,
    )


def make_table(feat_scaled):
    """[TBL_ROWS, TBL_STRIDE] f8 table; row n+1 payload = feat_scaled[n]."""
    t = np.zeros((TBL_ROWS, TBL_STRIDE), F8NP)
    t[1:N + 1, :F] = feat_scaled.astype(F8NP)
    return t


# ------------------------------------------------------------- raw dma_gather
def raw_dma_gather(gp, out_ap, in_ap, idxs_ap, num_idxs, elem_size, elem_step,
                   queue_num=0):
    """nc.gpsimd.dma_gather minus the 256B elem_size assert (transpose-only
    restriction in the Q7 ucode; the row STRIDE must be 256B-aligned and is)."""
    gp._assert_queue_num(queue_num)
    stride_bytes = elem_step * mybir.dt.size(in_ap.dtype)
    assert stride_bytes % 256 == 0
    _in_ap = gp.lower_ap_dma(in_ap, for_custom_bir_dma=True)
    _idxs_ap = gp.lower_ap(idxs_ap)
    _out_ap = gp.lower_ap(out_ap)
    return gp.add_instruction(
        mybir.InstDMAGatherAnt(
            name=gp.bass.get_next_instruction_name(),
            ins=[*_in_ap, _idxs_ap, gp.lower_val_access(gp.to_reg(num_idxs))],
            outs=[_out_ap],
            transpose=False,
            num_idxs=num_idxs,
            elem_size=elem_size,
            stride_bytes_256=stride_bytes // 256,
            gen_mode=0,
            single_packet=False,
            queue_num=queue_num,
            sbuf_tokens_per_rank=0,
            sbuf_free_dim_per_rank=0,
            sbuf_free_dim_pad_per_rank=0,
            sbuf_byte_offset=0,
        )
    )


# ---------------------------------------------------------------- device kernel
def build_kernel(layer, plan, has_b1=False, has_b2=False,
                 msg_bufs=3, wk_bufs=3, ps_bufs=2):
    """layer 1: table -> outT[128, 6272] fp16 = (W2^T relu(W1 agg))*dinv^2/s1
    layer 2: table -> yw[128, WPC*P] fp16 = relu(agg*dinv/s2 + b2), (w,f) cols
    """
    f32, f16, i16 = mybir.dt.float32, mybir.dt.float16, mybir.dt.int16
    kAg, kBg = plan["kAg"], plan["kBg"]
    soffA, soffB = plan["soffA"], plan["soffB"]
    SA, SB = int(soffA[-1]), int(soffB[-1])
    nc = bacc.Bacc("TRN2", debug=False)
    d = {}
    d["table"] = nc.dram_tensor("table", [TBL_ROWS, TBL_STRIDE], F8,
                                kind="ExternalInput").ap()
    d["idxA"] = nc.dram_tensor("idxA", [P, SA // 16], i16, kind="ExternalInput").ap()
    d["idxB"] = nc.dram_tensor("idxB", [P, SB // 16], i16, kind="ExternalInput").ap()
    d["ident"] = nc.dram_tensor("ident", [P, P], F8, kind="ExternalInput").ap()
    d["ones1"] = nc.dram_tensor("ones1", [1, P], f16, kind="ExternalInput").ap()
    if layer == 1:
        d["ident16"] = nc.dram_tensor("ident16", [P, P], f16,
                                      kind="ExternalInput").ap()
        d["w1t"] = nc.dram_tensor("w1t", [P, H], f16, kind="ExternalInput").ap()
        d["w2t"] = nc.dram_tensor("w2t", [P, H], f16, kind="ExternalInput").ap()
        # end multiplier row: dinv^2/s1 (b1==0) or dinv (general path)
        d["dendn"] = nc.dram_tensor("dendn", [1, WPC * P], f16,
                                    kind="ExternalInput").ap()
        if has_b1:
            d["dmidn"] = nc.dram_tensor("dmidn", [1, WPC * P], f16,
                                        kind="ExternalInput").ap()
            d["b1c"] = nc.dram_tensor("b1c", [P, H // P], f32,
                                      kind="ExternalInput").ap()
        out_d = nc.dram_tensor("out", [P, WPC * P], f16, kind="ExternalOutput").ap()
    else:
        d["ind"] = nc.dram_tensor("ind", [NG, NG * P], f16,
                                  kind="ExternalInput").ap()
        d["dinvg"] = nc.dram_tensor("dinvg", [NG, len(GROUPS) * P], f16,
                                    kind="ExternalInput").ap()
        if has_b2:
            d["u4"] = nc.dram_tensor("u4", [NG, len(GROUPS) * P], f16,
                                     kind="ExternalInput").ap()
            d["indb2"] = nc.dram_tensor("indb2", [NG, NG * P], f16,
                                        kind="ExternalInput").ap()
        out_d = nc.dram_tensor("out", [P, WPC * P], f16, kind="ExternalOutput").ap()

    Relu = mybir.ActivationFunctionType.Relu
    Copy = mybir.ActivationFunctionType.Copy

    if ps_bufs == 2:
        ps_bufs = 3 if layer == 1 else 5
    with tile.TileContext(nc) as tc:
        with (
            tc.tile_pool(name="cst", bufs=1) as cp,
            tc.tile_pool(name="msg", bufs=msg_bufs) as mp,
            tc.tile_pool(name="work", bufs=wk_bufs) as wp,
            tc.tile_pool(name="psA", bufs=ps_bufs, space="PSUM") as ppA,
            tc.tile_pool(name="psD", bufs=2, space="PSUM") as ppD,
            tc.tile_pool(name="psZ", bufs=1, space="PSUM") as ppZ,
        ):
            def load(name, shape, dtype):
                t = cp.tile(shape, dtype, tag=name)
                nc.sync.dma_start(out=t[:], in_=d[name][:])
                return t

            idxA_t = cp.tile([P, SA // 16], i16, tag="idxA")
            idxB_t = cp.tile([P, SB // 16], i16, tag="idxB")
            a1 = int(soffA[1]) // 16
            b1 = int(soffB[1]) // 16
            nc.sync.dma_start(out=idxA_t[:, 0:a1], in_=d["idxA"][:, 0:a1])
            nc.sync.dma_start(out=idxB_t[:, 0:b1], in_=d["idxB"][:, 0:b1])
            nc.sync.dma_start(out=idxA_t[:, a1:], in_=d["idxA"][:, a1:])
            nc.sync.dma_start(out=idxB_t[:, b1:], in_=d["idxB"][:, b1:])
            ident_t = load("ident", [P, P], F8)
            ones1_t = load("ones1", [1, P], f16)
            if layer == 1:
                ident16_t = load("ident16", [P, P], f16)
                w1t_t = load("w1t", [P, H], f16)
                w2t_t = load("w2t", [P, H], f16)
                dendn_t = load("dendn", [1, WPC * P], f16)
                if has_b1:
                    dmidn_t = load("dmidn", [1, WPC * P], f16)
                    b1c_t = load("b1c", [P, H // P], f32)
            else:
                ind_t = load("ind", [NG, NG * P], f16)
                dinvg_t = load("dinvg", [NG, len(GROUPS) * P], f16)
                if has_b2:
                    u4_t = load("u4", [NG, len(GROUPS) * P], f16)
                    indb2_t = load("indb2", [NG, NG * P], f16)

            # startup: build free-dim multiplier tables broadcast in SBUF
            def bcast_rows(dst_tile, src_row_t):
                # dst[p, c] = src_row[c] for all partitions
                for gi, (g0, nw) in enumerate(GROUPS):
                    c0, c1 = g0 * P, (g0 + nw) * P
                    bps = ppB.tile([P, NG * P], f32, tag="bps")
                    nc.tensor.matmul(out=bps[:, : c1 - c0], lhsT=ones1_t[:],
                                     rhs=src_row_t[:, c0:c1], start=True,
                                     stop=True)
                    nc.scalar.activation(out=dst_tile[:, c0:c1],
                                         in_=bps[:, : c1 - c0], func=Copy)

            if layer == 1:
                dend_b = cp.tile([P, WPC * P], f16, tag="dend_b")
                bcast_rows(dend_b, dendn_t)
                if has_b1:
                    dmid_b = cp.tile([P, WPC * P], f16, tag="dmid_b")
                    bcast_rows(dmid_b, dmidn_t)
            else:
                # dinvball2[d, (w,f)] = dinv(pos w, slot d)/s2, per group g
                dball2 = cp.tile([P, WPC * P], f16, tag="dball2")
                for gi, (g0, nw) in enumerate(GROUPS):
                    bps = ppB.tile([P, NG * P], f32, tag="bps")
                    nc.tensor.matmul(out=bps[:, : nw * P],
                                     lhsT=dinvg_t[0:nw, gi * P:(gi + 1) * P],
                                     rhs=ind_t[0:nw, : nw * P],
                                     start=True, stop=True)
                    nc.scalar.activation(out=dball2[:, g0 * P:(g0 + nw) * P],
                                         in_=bps[:, : nw * P], func=Copy)

            for gi, (g0, nw) in enumerate(GROUPS):
                wA = nw * P
                njA = int(soffA[gi + 1] - soffA[gi])
                njB = int(soffB[gi + 1] - soffB[gi])
                mtA = mp.tile([P, njA], F8, tag="mA")
                raw_dma_gather(
                    nc.gpsimd,
                    out_ap=mtA[:].rearrange("p (j e) -> p j e", e=F),
                    in_ap=d["table"][:, 0:F],
                    idxs_ap=idxA_t[:, int(soffA[gi]) // 16:int(soffA[gi + 1]) // 16],
                    num_idxs=njA, elem_size=F, elem_step=TBL_STRIDE,
                )
                mtB = mp.tile([P, njB], F8, tag="mB")
                raw_dma_gather(
                    nc.gpsimd,
                    out_ap=mtB[:].rearrange("p (j e) -> p j e", e=F),
                    in_ap=d["table"][B_OFF:TBL_ROWS, 0:F],
                    idxs_ap=idxB_t[:, int(soffB[gi]) // 16:int(soffB[gi + 1]) // 16],
                    num_idxs=njB, elem_size=F, elem_step=TBL_STRIDE,
                )

                agg4 = ppA.tile([P, NG * P], f32, tag="agg4")
                nmm = int(kAg[gi]) + int(kBg[gi])
                k = 0
                if layer == 2 and has_b2:
                    nc.tensor.matmul(out=agg4[:, :wA],
                                     lhsT=u4_t[0:nw, gi * P:(gi + 1) * P],
                                     rhs=indb2_t[0:nw, :wA],
                                     start=True, stop=False, skip_group_check=True)
                    k = 1
                    nmm += 1
                for mt, chw, choff in ((mtA, chwA[gi], choffA[gi]),
                                       (mtB, chwB[gi], choffB[gi])):
                    for c, mw in enumerate(chw):
                        nc.tensor.matmul(out=agg4[:, :mw * P], lhsT=ident_t[:],
                                         rhs=mt[:, int(choff[c]):int(choff[c + 1])],
                                         start=(k == 0), stop=(k == nmm - 1),
                                         skip_group_check=True)
                        k += 1

                if layer == 1:
                    # drain, transpose each window block, dense stages 4-wide
                    aggsb = wp.tile([P, NG * P], f16, tag="aggsb")
                    nc.vector.tensor_copy(out=aggsb[:, :wA], in_=agg4[:, :wA])
                    aggT = ppD.tile([P, NG * P], f16, tag="aggT")
                    for wi in range(nw):
                        nc.tensor.matmul(
                            out=aggT[:, wi * P:(wi + 1) * P],
                            lhsT=aggsb[:, wi * P:(wi + 1) * P],
                            rhs=ident16_t[:], start=True, stop=True,
                            is_transpose=True)
                    aggsT = wp.tile([P, NG * P], f16, tag="aggsT")
                    nc.vector.tensor_copy(out=aggsT[:, :wA], in_=aggT[:, :wA])
                    ht4 = wp.tile([P, (H // P) * NG * P], f16, tag="ht4")
                    for oc in range(H // P):
                        hps = ppD.tile([P, NG * P], f32, tag="h")
                        nc.tensor.matmul(out=hps[:, :wA],
                                         lhsT=w1t_t[:, oc * P:(oc + 1) * P],
                                         rhs=aggsT[:, :wA], start=True, stop=True)
                        hslc = ht4[:, oc * NG * P:oc * NG * P + wA]
                        if has_b1:
                            hsc = wp.tile([P, NG * P], f16, tag="hsc")
                            nc.vector.tensor_tensor(
                                out=hsc[:, :wA], in0=hps[:, :wA],
                                in1=dmid_b[:, g0 * P:(g0 + nw) * P],
                                op=mybir.AluOpType.mult)
                            nc.scalar.activation(out=hslc, in_=hsc[:, :wA],
                                                 func=Relu,
                                                 bias=b1c_t[:, oc:oc + 1])
                        else:
                            nc.scalar.activation(out=hslc, in_=hps[:, :wA],
                                                 func=Relu)
                    zT4 = ppZ.tile([P, NG * P], f32, tag="zT4")
                    for ic in range(H // P):
                        nc.tensor.matmul(
                            out=zT4[:, :wA],
                            lhsT=w2t_t[:, ic * P:(ic + 1) * P],
                            rhs=ht4[:, ic * NG * P:ic * NG * P + wA],
                            start=(ic == 0), stop=(ic == H // P - 1))
                    z4 = wp.tile([P, NG * P], f16, tag="z4")
                    nc.vector.tensor_tensor(
                        out=z4[:, :wA], in0=zT4[:, :wA],
                        in1=dend_b[:, g0 * P:(g0 + nw) * P],
                        op=mybir.AluOpType.mult)
                    nc.sync.dma_start(out=out_d[:, g0 * P:(g0 + nw) * P],
                                      in_=z4[:, :wA])
                else:
                    u4s = wp.tile([P, NG * P], f16, tag="u4s")
                    nc.vector.tensor_tensor(
                        out=u4s[:, :wA], in0=agg4[:, :wA],
                        in1=dball2[:, g0 * P:(g0 + nw) * P],
                        op=mybir.AluOpType.mult)
                    y4 = wp.tile([P, NG * P], f16, tag="y4")
                    nc.scalar.activation(out=y4[:, :wA], in_=u4s[:, :wA],
                                         func=Relu)
                    nc.sync.dma_start(out=out_d[:, g0 * P:(g0 + nw) * P],
                                      in_=y4[:, :wA])

    nc.compile()
    return nc


# ---------------------------------------------------------------- entry point
def _in_maps(plan, layer, table, W1=None, b1=None, W2=None, b2=None,
             s1=1.0, s2=1.0):
    ident = np.eye(P, dtype=F8NP)
    has_b1 = b1 is not None and np.any(b1)
    has_b2 = b2 is not None and np.any(b2)
    NGRP = len(GROUPS)
    maps = []
    for c in range(NCORES):
        m = dict(table=table, ident=ident,
                 idxA=plan["idxA"][c], idxB=plan["idxB"][c],
                 ones1=np.ones((1, P), np.float16))
        dr = plan["dinvrow"][c]  # [WPC*P], position-major
        if layer == 1:
            m["ident16"] = np.eye(P, dtype=np.float16)
            m["w1t"] = W1.T.astype(np.float16).copy()
            m["w2t"] = np.concatenate(
                [W2[:, c0 * P:(c0 + 1) * P].T for c0 in range(H // P)], axis=1
            ).astype(np.float16).copy()
            if has_b1:
                m["dendn"] = dr.astype(np.float16)[None, :]
                m["dmidn"] = (dr / s1).astype(np.float16)[None, :]
                m["b1c"] = b1.reshape(H // P, P).T.astype(np.float32).copy()
            else:
                m["dendn"] = (dr * dr / s1).astype(np.float16)[None, :]
        else:
            # dinvg[j, g*P+d] = dinv(pos g*NG+j, slot d)/s2
            dg = np.zeros((NG, NGRP * P), np.float32)
            for gi, (g0, nw) in enumerate(GROUPS):
                for j in range(nw):
                    dg[j, gi * P:(gi + 1) * P] = dr[(g0 + j) * P:(g0 + j + 1) * P]
            m["dinvg"] = (dg / s2).astype(np.float16)
            ind = np.zeros((NG, NG * P), np.float16)
            for j in range(NG):
                ind[j, j * P:(j + 1) * P] = 1.0
            m["ind"] = ind
            if has_b2:
                ug = np.where(dg > 0, s2 / np.maximum(dg, 1e-9), 0.0)
                m["u4"] = ug.astype(np.float16)
                indb2 = np.zeros((NG, NG * P), np.float16)
                for j in range(NG):
                    indb2[j, j * P:(j + 1) * P] = b2.astype(np.float16)
                m["indb2"] = indb2
        maps.append(m)
    return maps


def decode_l1(plan, outs):
    allo = np.stack(outs)  # [C, 128f, WPC*P cols]
    return allo[plan["node_core"], :, plan["node_row"]]  # [N, F]


def decode_l2(plan, outs):
    allo = np.stack(outs)  # [C, 128d, (w,f) cols]
    return allo[plan["node_core"][:, None], plan["node_p"][:, None],
                plan["node_w"][:, None] * P + np.arange(F)[None, :]]


def kernel(**inputs):
    x = np.asarray(inputs["x"], np.float32)
    edge_index = np.asarray(inputs["edge_index"])
    W1 = np.asarray(inputs["W1"], np.float32)
    b1 = np.asarray(inputs["b1"], np.float32)
    W2 = np.asarray(inputs["W2"], np.float32)
    b2 = np.asarray(inputs["b2"], np.float32)
    has_b1, has_b2 = bool(np.any(b1)), bool(np.any(b2))

    plan = build_plan(edge_index)
    nc1 = build_kernel(1, plan, has_b1=has_b1)
    nc2 = build_kernel(2, plan, has_b2=has_b2)

    t0 = x * plan["dinv"][:, None]
    s1 = float(0.995 * F8MAX / max(np.abs(t0).max(), 1e-9))
    r1 = run_bass_kernel_spmd(
        nc1, _in_maps(plan, 1, make_table(t0 * s1), W1=W1, b1=b1, W2=W2, s1=s1),
        core_ids=list(range(NCORES)))
    zt = decode_l1(plan, [np.asarray(r1.results[c]["out"], np.float32)
                          for c in range(NCORES)])
    s2 = float(0.995 * F8MAX / max(np.abs(zt).max(), 1e-9))
    r2 = run_bass_kernel_spmd(
        nc2, _in_maps(plan, 2, make_table(zt * s2), b2=b2, s2=s2),
        core_ids=list(range(NCORES)))
    y = decode_l2(plan, [np.asarray(r2.results[c]["out"], np.float32)
                         for c in range(NCORES)])

    last_run_info["exec_time_ns"] = [r1.exec_time_ns, r2.exec_time_ns]
    last_run_info["ncs"] = (nc1, nc2)
    last_run_info["plan"] = plan
    return y.astype(np.float32)


# revision 24
# speedup vs baseline: 1.6962x; 1.0285x over previous
"""Trainium2 Bass kernel for 2-layer GCN (N=50000, E=600000, 128->512->128).

Strategy (8 NeuronCores, graph/data parallel over destination nodes):
  - Aggregate-then-transform: segment_sum commutes with the linear layers, so
    both layers aggregate 128-wide features.  Symmetric normalization is
    separable: table rows are pre-scaled by dinv[src], aggregates post-scaled
    by dinv[dst].
  - Identity-S packing: nodes are greedily packed into 392 windows of 128
    destination nodes minimizing sum(maxA+maxB) chunk counts; window chunk c
    holds, at slot p, the c-th incoming edge of the window's p-th node, so the
    PE aggregates each chunk against a constant identity matrix (no per-window
    one-hot build).  Windows are grouped 8-to-a-position across cores (one
    SPMD shape) and 4-positions-to-a-group so each chunk matmul carries a
    512-wide moving operand: PE sequencer issue rate, not FLOPs, is the
    limiter.
  - fp8(e3m4) gather table, rows on a 256B stride: dma_gather is emitted
    directly (the 256B elem_size assert is a transpose-only Q7 restriction;
    the stride must be 256B-aligned and is), so each edge moves 128 bytes.
    Tables are optimally scaled per layer on the host; inverse scales fold
    into the dinv normalization multipliers.
  - Layer 1 on-chip: agg4[d,(wi,f)] -> transpose -> @W1^T, relu -> @W2^T ->
    zT*(dinv^2/s1) -> fp16 (b1==0 lets dinv commute past relu; a general b1
    path applies dinv/s1 before the biased relu).  Host rescales z into the
    layer-2 table.  Layer 2: agg4 -> relu(agg*dinv/s2 + b2) -> y fp16.
"""

import numpy as np

import concourse.bacc as bacc
import concourse.mybir as mybir
import concourse.tile as tile
from concourse.bass_utils import run_bass_kernel_spmd

# problem constants (hardcoded per contract)
N = 50000
E = 600000
F = 128          # in/out feature dim
H = 512          # hidden dim
P = 128
NCORES = 8
WPC = 49                  # window positions per core
BINS = NCORES * WPC       # 392
ROWS_PER_CORE = WPC * P   # 6272 output rows per core (>= 6250 real)
TBL_ROWS = N + 2          # zero row at 0 and N+1
TBL_STRIDE = 256          # fp8 elems between rows (256B stride, 128B payload)
A_MAX_SRC = 32766         # srcs <= this go to range A (idx = src+1 <= 32767)
B_OFF = 17234             # range B table view starts at this row
B_PAD_IDX = 32767         # row N+1 (zero) relative to B view
NG = 4                    # positions per matmul group (512-wide moving ops)

F8 = mybir.dt.float8e3
F8NP = mybir.dt.np(F8)    # ml_dtypes.float8_e3m4
F8MAX = 15.5

GROUPS = [(g0, min(NG, WPC - g0)) for g0 in range(0, WPC, NG)]  # (start, width)

last_run_info = {}


# ---------------------------------------------------------------- host planner
def build_plan(edge_index):
    src = np.asarray(edge_index[0]).astype(np.int64)
    dst = np.asarray(edge_index[1]).astype(np.int64)

    deg = np.bincount(dst, minlength=N).astype(np.int64) + 1  # + self loop
    dinv = (1.0 / np.sqrt(deg)).astype(np.float32)

    iota = np.arange(N, dtype=np.int64)
    e_isa = src <= A_MAX_SRC
    a_cnt = np.bincount(dst[e_isa], minlength=N) + (iota <= A_MAX_SRC)
    b_cnt = np.bincount(dst[~e_isa], minlength=N) + (iota > A_MAX_SRC)

    # stage 1: greedy-pack nodes into 392 windows minimizing sum(maxA+maxB)
    order = np.lexsort((-b_cnt, -(a_cnt + b_cnt)))  # heaviest first
    ka = np.zeros(BINS)
    kb = np.zeros(BINS)
    cnt = np.zeros(BINS)
    binof = np.zeros(N, np.int64)
    for n in order:
        an, bn = a_cnt[n], b_cnt[n]
        pen = np.maximum(an - ka, 0) + np.maximum(bn - kb, 0)
        pen[cnt >= P] = 1e9
        j = int(np.argmin(pen * 100000 - cnt))
        binof[n] = j
        ka[j] = max(ka[j], an)
        kb[j] = max(kb[j], bn)
        cnt[j] += 1

    # stage 2: group the 392 windows into 49 SPMD positions of 8 cores
    worder = np.lexsort((-kb, -ka))
    gof = np.zeros(BINS, np.int64)
    gof[worder] = np.arange(BINS) // NCORES

    def group_cost(g):
        gka = np.zeros(WPC)
        gkb = np.zeros(WPC)
        np.maximum.at(gka, g, ka)
        np.maximum.at(gkb, g, kb)
        return np.maximum(gka, 1).sum() + np.maximum(gkb, 1).sum()

    rng = np.random.default_rng(0)
    cur = group_cost(gof)
    for _ in range(30000):
        i, j = rng.integers(0, BINS, 2)
        if gof[i] == gof[j]:
            continue
        gof[i], gof[j] = gof[j], gof[i]
        c2 = group_cost(gof)
        if c2 < cur:
            cur = c2
        else:
            gof[i], gof[j] = gof[j], gof[i]

    gka = np.zeros(WPC)
    gkb = np.zeros(WPC)
    np.maximum.at(gka, gof, ka)
    np.maximum.at(gkb, gof, kb)
    kA = np.maximum(gka, 1).astype(np.int64)
    kB = np.maximum(gkb, 1).astype(np.int64)

    # re-sort positions by profile, then DP-partition into groups of width<=NG
    # (variable width trades gather padding against PE instruction count),
    # then swap-refine memberships
    porder = np.lexsort((-kB, -kA))
    a_s, b_s = kA[porder], kB[porder]
    LAM = 100.0
    INF = float("inf")
    best = [INF] * (WPC + 1)
    best[0] = 0.0
    chw = [0] * (WPC + 1)
    for i in range(1, WPC + 1):
        for w in range(1, NG + 1):
            if i - w < 0:
                break
            c = best[i - w] + (w * P + LAM) * (a_s[i - w:i].max()
                                               + b_s[i - w:i].max())
            if c < best[i]:
                best[i] = c
                chw[i] = w
    memb = []
    i = WPC
    while i > 0:
        w = chw[i]
        memb.append(list(range(i - w, i)))
        i -= w
    memb.reverse()

    def gcost(m):
        return len(m) * (a_s[m].max() + b_s[m].max())

    gcosts = [gcost(np.array(m)) for m in memb]
    for _ in range(20000):
        gi, gj = rng.integers(0, len(memb), 2)
        if gi == gj:
            continue
        mi, mj = memb[gi], memb[gj]
        ii, jj = rng.integers(0, len(mi)), rng.integers(0, len(mj))
        mi[ii], mj[jj] = mj[jj], mi[ii]
        c1, c2 = gcost(np.array(mi)), gcost(np.array(mj))
        if c1 + c2 < gcosts[gi] + gcosts[gj]:
            gcosts[gi], gcosts[gj] = c1, c2
        else:
            mi[ii], mj[jj] = mj[jj], mi[ii]

    # relabel positions so each group's positions are consecutive,
    # each group internally sorted by (kA desc, kB desc) so chunk widths
    # are (near-)prefixes
    for m in memb:
        m.sort(key=lambda si: (-a_s[si], -b_s[si]))
    flat = [s for m in memb for s in m]       # sorted-space index -> order
    newpos_of_sorted = np.empty(WPC, np.int64)
    newpos_of_sorted[flat] = np.arange(WPC)
    pos_new = np.empty(WPC, np.int64)
    pos_new[porder] = newpos_of_sorted        # original pos -> new pos
    gof = pos_new[gof]
    kA2 = np.empty(WPC, np.int64)
    kB2 = np.empty(WPC, np.int64)
    kA2[newpos_of_sorted] = a_s
    kB2[newpos_of_sorted] = b_s
    kA, kB = kA2, kB2
    groups = []
    off = 0
    for m in memb:
        groups.append((off, len(m)))
        off += len(m)

    # per-group variable-width chunks: chunk c spans the first mA_c windows
    # (A side, prefix by sort) / up to the last window with kB>c (B side)
    kAg = np.array([int(kA[g0:g0 + nw].max()) for g0, nw in groups])
    kBg = np.array([int(kB[g0:g0 + nw].max()) for g0, nw in groups])
    chwA, choffA, chwB, choffB = [], [], [], []
    for gi, (g0, nw) in enumerate(groups):
        wa = [int(np.sum(kA[g0:g0 + nw] > c)) for c in range(kAg[gi])]
        wb = [int(np.max(np.nonzero(kB[g0:g0 + nw] > c)[0])) + 1
              for c in range(kBg[gi])]
        chwA.append(wa)
        chwB.append(wb)
        choffA.append(np.concatenate([[0], np.cumsum(wa)]).astype(np.int64) * P)
        choffB.append(np.concatenate([[0], np.cumsum(wb)]).astype(np.int64) * P)
    soffA = np.concatenate([[0], np.cumsum([o[-1] for o in choffA])]).astype(
        np.int64)
    soffB = np.concatenate([[0], np.cumsum([o[-1] for o in choffB])]).astype(
        np.int64)
    SA, SB = int(soffA[-1]), int(soffB[-1])

    # node -> (core, position, slot)
    corein = np.zeros(BINS, np.int64)
    for w in range(WPC):
        bw = np.where(gof == w)[0]
        corein[bw] = np.arange(len(bw))
    node_core = corein[binof]
    node_w = gof[binof]
    bsort = np.argsort(binof, kind="stable")
    bcounts = np.bincount(binof, minlength=BINS)
    bstarts = np.concatenate([[0], np.cumsum(bcounts)[:-1]])
    node_p = np.empty(N, np.int64)
    node_p[bsort] = np.arange(N) - bstarts[binof[bsort]]
    node_row = node_w * P + node_p

    # edges + self loops, ranked within (dst node, range class)
    es = np.concatenate([src, iota])
    ed = np.concatenate([dst, iota])
    ecls = (es > A_MAX_SRC).astype(np.int64)
    key = ed * 2 + ecls
    eorder = np.argsort(key, kind="stable")
    ks = key[eorder]
    counts = np.bincount(key, minlength=2 * N)
    starts = np.concatenate([[0], np.cumsum(counts)[:-1]])
    rank = np.arange(len(es)) - starts[ks]
    se, de = es[eorder], ed[eorder]

    pos2g = np.empty(WPC, np.int64)
    pos2wi = np.empty(WPC, np.int64)
    for gi, (g0, nw) in enumerate(groups):
        pos2g[g0:g0 + nw] = gi
        pos2wi[g0:g0 + nw] = np.arange(nw)
    c_, w_, p_ = node_core[de], node_w[de], node_p[de]
    g_, wi_ = pos2g[w_], pos2wi[w_]
    flatoffA = np.zeros((len(groups), int(kAg.max()) + 1), np.int64)
    flatoffB = np.zeros((len(groups), int(kBg.max()) + 1), np.int64)
    for gi in range(len(groups)):
        flatoffA[gi, :len(choffA[gi])] = soffA[gi] + choffA[gi]
        flatoffB[gi, :len(choffB[gi])] = soffB[gi] + choffB[gi]
    mA = ks % 2 == 0
    idxA_flat = np.zeros(NCORES * SA, np.int16)
    linA = (c_[mA] * SA + flatoffA[g_[mA], rank[mA]] + wi_[mA] * P + p_[mA])
    idxA_flat[linA] = (se[mA] + 1).astype(np.int16)
    mB = ~mA
    idxB_flat = np.full(NCORES * SB, B_PAD_IDX, np.int16)
    linB = (c_[mB] * SB + flatoffB[g_[mB], rank[mB]] + wi_[mB] * P + p_[mB])
    idxB_flat[linB] = (se[mB] + 1 - B_OFF).astype(np.int16)

    def wrap(flat, stot):  # [NCORES, stot] -> [NCORES, 128, stot/16]
        a = flat.reshape(NCORES, stot // 16, 16)
        a = np.transpose(a, (0, 2, 1))
        return np.tile(a, (1, 8, 1)).copy()

    dinvrow = np.zeros((NCORES, ROWS_PER_CORE), np.float32)
    dinvrow[node_core, node_row] = dinv

    return dict(
        kA=kA, kB=kB, kAg=kAg, kBg=kBg, soffA=soffA, soffB=soffB, dinv=dinv,
        chwA=chwA, chwB=chwB, choffA=choffA, choffB=choffB,
        groups=groups, idxA=wrap(idxA_flat, SA), idxB=wrap(idxB_flat, SB),
        dinvrow=dinvrow, node_core=node_core, node_row=node_row,
        node_w=node_w, node_p=node_p,
    )


def make_table(feat_scaled):
    """[TBL_ROWS, TBL_STRIDE] f8 table; row n+1 payload = feat_scaled[n]."""
    t = np.zeros((TBL_ROWS, TBL_STRIDE), F8NP)
    t[1:N + 1, :F] = feat_scaled.astype(F8NP)
    return t


# ------------------------------------------------------------- raw dma_gather
def raw_dma_gather(gp, out_ap, in_ap, idxs_ap, num_idxs, elem_size, elem_step,
                   queue_num=0):
    """nc.gpsimd.dma_gather minus the 256B elem_size assert (transpose-only
    restriction in the Q7 ucode; the row STRIDE must be 256B-aligned and is)."""
    gp._assert_queue_num(queue_num)
    stride_bytes = elem_step * mybir.dt.size(in_ap.dtype)
    assert stride_bytes % 256 == 0
    _in_ap = gp.lower_ap_dma(in_ap, for_custom_bir_dma=True)
    _idxs_ap = gp.lower_ap(idxs_ap)
    _out_ap = gp.lower_ap(out_ap)
    return gp.add_instruction(
        mybir.InstDMAGatherAnt(
            name=gp.bass.get_next_instruction_name(),
            ins=[*_in_ap, _idxs_ap, gp.lower_val_access(gp.to_reg(num_idxs))],
            outs=[_out_ap],
            transpose=False,
            num_idxs=num_idxs,
            elem_size=elem_size,
            stride_bytes_256=stride_bytes // 256,
            gen_mode=0,
            single_packet=False,
            queue_num=queue_num,
            sbuf_tokens_per_rank=0,
            sbuf_free_dim_per_rank=0,
            sbuf_free_dim_pad_per_rank=0,
            sbuf_byte_offset=0,
        )
    )


# ---------------------------------------------------------------- device kernel
def build_kernel(layer, plan, has_b1=False, has_b2=False,
                 msg_bufs=3, wk_bufs=3, ps_bufs=2):
    """layer 1: table -> outT[128, 6272] fp16 = (W2^T relu(W1 agg))*dinv^2/s1
    layer 2: table -> yw[128, WPC*P] fp16 = relu(agg*dinv/s2 + b2), (w,f) cols
    """
    f32, f16, i16 = mybir.dt.float32, mybir.dt.float16, mybir.dt.int16
    kAg, kBg = plan["kAg"], plan["kBg"]
    soffA, soffB = plan["soffA"], plan["soffB"]
    SA, SB = int(soffA[-1]), int(soffB[-1])
    nc = bacc.Bacc("TRN2", debug=False)
    d = {}
    d["table"] = nc.dram_tensor("table", [TBL_ROWS, TBL_STRIDE], F8,
                                kind="ExternalInput").ap()
    d["idxA"] = nc.dram_tensor("idxA", [P, SA // 16], i16, kind="ExternalInput").ap()
    d["idxB"] = nc.dram_tensor("idxB", [P, SB // 16], i16, kind="ExternalInput").ap()
    d["ident"] = nc.dram_tensor("ident", [P, P], F8, kind="ExternalInput").ap()
    d["ones1"] = nc.dram_tensor("ones1", [1, P], f16, kind="ExternalInput").ap()
    if layer == 1:
        d["ident16"] = nc.dram_tensor("ident16", [P, P], f16,
                                      kind="ExternalInput").ap()
        d["w1t"] = nc.dram_tensor("w1t", [P, H], f16, kind="ExternalInput").ap()
        d["w2t"] = nc.dram_tensor("w2t", [P, H], f16, kind="ExternalInput").ap()
        # end multiplier row: dinv^2/s1 (b1==0) or dinv (general path)
        d["dendn"] = nc.dram_tensor("dendn", [1, WPC * P], f16,
                                    kind="ExternalInput").ap()
        if has_b1:
            d["dmidn"] = nc.dram_tensor("dmidn", [1, WPC * P], f16,
                                        kind="ExternalInput").ap()
            d["b1c"] = nc.dram_tensor("b1c", [P, H // P], f32,
                                      kind="ExternalInput").ap()
        out_d = nc.dram_tensor("out", [P, WPC * P], f16, kind="ExternalOutput").ap()
    else:
        d["ind"] = nc.dram_tensor("ind", [NG, NG * P], f16,
                                  kind="ExternalInput").ap()
        d["dinvg"] = nc.dram_tensor("dinvg", [NG, len(GROUPS) * P], f16,
                                    kind="ExternalInput").ap()
        if has_b2:
            d["u4"] = nc.dram_tensor("u4", [NG, len(GROUPS) * P], f16,
                                     kind="ExternalInput").ap()
            d["indb2"] = nc.dram_tensor("indb2", [NG, NG * P], f16,
                                        kind="ExternalInput").ap()
        out_d = nc.dram_tensor("out", [P, WPC * P], f16, kind="ExternalOutput").ap()

    Relu = mybir.ActivationFunctionType.Relu
    Copy = mybir.ActivationFunctionType.Copy

    if ps_bufs == 2:
        ps_bufs = 3 if layer == 1 else 5
    with tile.TileContext(nc) as tc:
        with (
            tc.tile_pool(name="cst", bufs=1) as cp,
            tc.tile_pool(name="msg", bufs=msg_bufs) as mp,
            tc.tile_pool(name="work", bufs=wk_bufs) as wp,
            tc.tile_pool(name="psA", bufs=ps_bufs, space="PSUM") as ppA,
            tc.tile_pool(name="psD", bufs=2, space="PSUM") as ppD,
            tc.tile_pool(name="psZ", bufs=1, space="PSUM") as ppZ,
        ):
            def load(name, shape, dtype):
                t = cp.tile(shape, dtype, tag=name)
                nc.sync.dma_start(out=t[:], in_=d[name][:])
                return t

            idxA_t = cp.tile([P, SA // 16], i16, tag="idxA")
            idxB_t = cp.tile([P, SB // 16], i16, tag="idxB")
            a1 = int(soffA[1]) // 16
            b1 = int(soffB[1]) // 16
            nc.sync.dma_start(out=idxA_t[:, 0:a1], in_=d["idxA"][:, 0:a1])
            nc.sync.dma_start(out=idxB_t[:, 0:b1], in_=d["idxB"][:, 0:b1])
            nc.sync.dma_start(out=idxA_t[:, a1:], in_=d["idxA"][:, a1:])
            nc.sync.dma_start(out=idxB_t[:, b1:], in_=d["idxB"][:, b1:])
            ident_t = load("ident", [P, P], F8)
            ones1_t = load("ones1", [1, P], f16)
            if layer == 1:
                ident16_t = load("ident16", [P, P], f16)
                w1t_t = load("w1t", [P, H], f16)
                w2t_t = load("w2t", [P, H], f16)
                dendn_t = load("dendn", [1, WPC * P], f16)
                if has_b1:
                    dmidn_t = load("dmidn", [1, WPC * P], f16)
                    b1c_t = load("b1c", [P, H // P], f32)
            else:
                ind_t = load("ind", [NG, NG * P], f16)
                dinvg_t = load("dinvg", [NG, len(GROUPS) * P], f16)
                if has_b2:
                    u4_t = load("u4", [NG, len(GROUPS) * P], f16)
                    indb2_t = load("indb2", [NG, NG * P], f16)

            # startup: build free-dim multiplier tables broadcast in SBUF
            def bcast_rows(dst_tile, src_row_t):
                # dst[p, c] = src_row[c] for all partitions
                for gi, (g0, nw) in enumerate(GROUPS):
                    c0, c1 = g0 * P, (g0 + nw) * P
                    bps = ppB.tile([P, NG * P], f32, tag="bps")
                    nc.tensor.matmul(out=bps[:, : c1 - c0], lhsT=ones1_t[:],
                                     rhs=src_row_t[:, c0:c1], start=True,
                                     stop=True)
                    nc.scalar.activation(out=dst_tile[:, c0:c1],
                                         in_=bps[:, : c1 - c0], func=Copy)

            if layer == 1:
                dend_b = cp.tile([P, WPC * P], f16, tag="dend_b")
                bcast_rows(dend_b, dendn_t)
                if has_b1:
                    dmid_b = cp.tile([P, WPC * P], f16, tag="dmid_b")
                    bcast_rows(dmid_b, dmidn_t)
            else:
                # dinvball2[d, (w,f)] = dinv(pos w, slot d)/s2, per group g
                dball2 = cp.tile([P, WPC * P], f16, tag="dball2")
                for gi, (g0, nw) in enumerate(GROUPS):
                    bps = ppB.tile([P, NG * P], f32, tag="bps")
                    nc.tensor.matmul(out=bps[:, : nw * P],
                                     lhsT=dinvg_t[0:nw, gi * P:(gi + 1) * P],
                                     rhs=ind_t[0:nw, : nw * P],
                                     start=True, stop=True)
                    nc.scalar.activation(out=dball2[:, g0 * P:(g0 + nw) * P],
                                         in_=bps[:, : nw * P], func=Copy)

            for gi, (g0, nw) in enumerate(GROUPS):
                wA = nw * P
                njA = int(soffA[gi + 1] - soffA[gi])
                njB = int(soffB[gi + 1] - soffB[gi])
                mtA = mp.tile([P, njA], F8, tag="mA")
                raw_dma_gather(
                    nc.gpsimd,
                    out_ap=mtA[:].rearrange("p (j e) -> p j e", e=F),
                    in_ap=d["table"][:, 0:F],
                    idxs_ap=idxA_t[:, int(soffA[gi]) // 16:int(soffA[gi + 1]) // 16],
                    num_idxs=njA, elem_size=F, elem_step=TBL_STRIDE,
                )
                mtB = mp.tile([P, njB], F8, tag="mB")
                raw_dma_gather(
                    nc.gpsimd,
                    out_ap=mtB[:].rearrange("p (j e) -> p j e", e=F),
                    in_ap=d["table"][B_OFF:TBL_ROWS, 0:F],
                    idxs_ap=idxB_t[:, int(soffB[gi]) // 16:int(soffB[gi + 1]) // 16],
                    num_idxs=njB, elem_size=F, elem_step=TBL_STRIDE,
                )

                agg4 = ppA.tile([P, NG * P], f32, tag="agg4")
                nmm = int(kAg[gi]) + int(kBg[gi])
                k = 0
                if layer == 2 and has_b2:
                    nc.tensor.matmul(out=agg4[:, :wA],
                                     lhsT=u4_t[0:nw, gi * P:(gi + 1) * P],
                                     rhs=indb2_t[0:nw, :wA],
                                     start=True, stop=False, skip_group_check=True)
                    k = 1
                    nmm += 1
                for mt, chw, choff in ((mtA, chwA[gi], choffA[gi]),
                                       (mtB, chwB[gi], choffB[gi])):
                    for c, mw in enumerate(chw):
                        nc.tensor.matmul(out=agg4[:, :mw * P], lhsT=ident_t[:],
                                         rhs=mt[:, int(choff[c]):int(choff[c + 1])],
                                         start=(k == 0), stop=(k == nmm - 1),
                                         skip_group_check=True)
                        k += 1

                if layer == 1:
                    # drain, transpose each window block, dense stages 4-wide
                    aggsb = wp.tile([P, NG * P], f16, tag="aggsb")
                    nc.vector.tensor_copy(out=aggsb[:, :wA], in_=agg4[:, :wA])
                    aggT = ppD.tile([P, NG * P], f16, tag="aggT")
                    for wi in range(nw):
                        nc.tensor.matmul(
                            out=aggT[:, wi * P:(wi + 1) * P],
                            lhsT=aggsb[:, wi * P:(wi + 1) * P],
                            rhs=ident16_t[:], start=True, stop=True,
                            is_transpose=True)
                    aggsT = wp.tile([P, NG * P], f16, tag="aggsT")
                    nc.vector.tensor_copy(out=aggsT[:, :wA], in_=aggT[:, :wA])
                    ht4 = wp.tile([P, (H // P) * NG * P], f16, tag="ht4")
                    for oc in range(H // P):
                        hps = ppD.tile([P, NG * P], f32, tag="h")
                        nc.tensor.matmul(out=hps[:, :wA],
                                         lhsT=w1t_t[:, oc * P:(oc + 1) * P],
                                         rhs=aggsT[:, :wA], start=True, stop=True)
                        hslc = ht4[:, oc * NG * P:oc * NG * P + wA]
                        if has_b1:
                            hsc = wp.tile([P, NG * P], f16, tag="hsc")
                            nc.vector.tensor_tensor(
                                out=hsc[:, :wA], in0=hps[:, :wA],
                                in1=dmid_b[:, g0 * P:(g0 + nw) * P],
                                op=mybir.AluOpType.mult)
                            nc.scalar.activation(out=hslc, in_=hsc[:, :wA],
                                                 func=Relu,
                                                 bias=b1c_t[:, oc:oc + 1])
                        else:
                            nc.scalar.activation(out=hslc, in_=hps[:, :wA],
                                                 func=Relu)
                    zT4 = ppZ.tile([P, NG * P], f32, tag="zT4")
                    for ic in range(H // P):
                        nc.tensor.matmul(
                            out=zT4[:, :wA],
                            lhsT=w2t_t[:, ic * P:(ic + 1) * P],
                            rhs=ht4[:, ic * NG * P:ic * NG * P + wA],
                            start=(ic == 0), stop=(ic == H // P - 1))
                    z4 = wp.tile([P, NG * P], f16, tag="z4")
                    nc.vector.tensor_tensor(
                        out=z4[:, :wA], in0=zT4[:, :wA],
                        in1=dend_b[:, g0 * P:(g0 + nw) * P],
                        op=mybir.AluOpType.mult)
                    nc.sync.dma_start(out=out_d[:, g0 * P:(g0 + nw) * P],
                                      in_=z4[:, :wA])
                else:
                    u4s = wp.tile([P, NG * P], f16, tag="u4s")
                    nc.vector.tensor_tensor(
                        out=u4s[:, :wA], in0=agg4[:, :wA],
                        in1=dball2[:, g0 * P:(g0 + nw) * P],
                        op=mybir.AluOpType.mult)
                    y4 = wp.tile([P, NG * P], f16, tag="y4")
                    nc.scalar.activation(out=y4[:, :wA], in_=u4s[:, :wA],
                                         func=Relu)
                    nc.sync.dma_start(out=out_d[:, g0 * P:(g0 + nw) * P],
                                      in_=y4[:, :wA])

    nc.compile()
    return nc


# ---------------------------------------------------------------- entry point
def _in_maps(plan, layer, table, W1=None, b1=None, W2=None, b2=None,
             s1=1.0, s2=1.0):
    ident = np.eye(P, dtype=F8NP)
    has_b1 = b1 is not None and np.any(b1)
    has_b2 = b2 is not None and np.any(b2)
    NGRP = len(GROUPS)
    maps = []
    for c in range(NCORES):
        m = dict(table=table, ident=ident,
                 idxA=plan["idxA"][c], idxB=plan["idxB"][c],
                 ones1=np.ones((1, P), np.float16))
        dr = plan["dinvrow"][c]  # [WPC*P], position-major
        if layer == 1:
            m["ident16"] = np.eye(P, dtype=np.float16)
            m["w1t"] = W1.T.astype(np.float16).copy()
            m["w2t"] = np.concatenate(
                [W2[:, c0 * P:(c0 + 1) * P].T for c0 in range(H // P)], axis=1
            ).astype(np.float16).copy()
            if has_b1:
                m["dendn"] = dr.astype(np.float16)[None, :]
                m["dmidn"] = (dr / s1).astype(np.float16)[None, :]
                m["b1c"] = b1.reshape(H // P, P).T.astype(np.float32).copy()
            else:
                m["dendn"] = (dr * dr / s1).astype(np.float16)[None, :]
        else:
            # dinvg[j, g*P+d] = dinv(pos g*NG+j, slot d)/s2
            dg = np.zeros((NG, NGRP * P), np.float32)
            for gi, (g0, nw) in enumerate(GROUPS):
                for j in range(nw):
                    dg[j, gi * P:(gi + 1) * P] = dr[(g0 + j) * P:(g0 + j + 1) * P]
            m["dinvg"] = (dg / s2).astype(np.float16)
            ind = np.zeros((NG, NG * P), np.float16)
            for j in range(NG):
                ind[j, j * P:(j + 1) * P] = 1.0
            m["ind"] = ind
            if has_b2:
                ug = np.where(dg > 0, s2 / np.maximum(dg, 1e-9), 0.0)
                m["u4"] = ug.astype(np.float16)
                indb2 = np.zeros((NG, NG * P), np.float16)
                for j in range(NG):
                    indb2[j, j * P:(j + 1) * P] = b2.astype(np.float16)
                m["indb2"] = indb2
        maps.append(m)
    return maps


def decode_l1(plan, outs):
    allo = np.stack(outs)  # [C, 128f, WPC*P cols]
    return allo[plan["node_core"], :, plan["node_row"]]  # [N, F]


def decode_l2(plan, outs):
    allo = np.stack(outs)  # [C, 128d, (w,f) cols]
    return allo[plan["node_core"][:, None], plan["node_p"][:, None],
                plan["node_w"][:, None] * P + np.arange(F)[None, :]]


def kernel(**inputs):
    x = np.asarray(inputs["x"], np.float32)
    edge_index = np.asarray(inputs["edge_index"])
    W1 = np.asarray(inputs["W1"], np.float32)
    b1 = np.asarray(inputs["b1"], np.float32)
    W2 = np.asarray(inputs["W2"], np.float32)
    b2 = np.asarray(inputs["b2"], np.float32)
    has_b1, has_b2 = bool(np.any(b1)), bool(np.any(b2))

    plan = build_plan(edge_index)
    nc1 = build_kernel(1, plan, has_b1=has_b1)
    nc2 = build_kernel(2, plan, has_b2=has_b2)

    t0 = x * plan["dinv"][:, None]
    s1 = float(0.995 * F8MAX / max(np.abs(t0).max(), 1e-9))
    r1 = run_bass_kernel_spmd(
        nc1, _in_maps(plan, 1, make_table(t0 * s1), W1=W1, b1=b1, W2=W2, s1=s1),
        core_ids=list(range(NCORES)))
    zt = decode_l1(plan, [np.asarray(r1.results[c]["out"], np.float32)
                          for c in range(NCORES)])
    s2 = float(0.995 * F8MAX / max(np.abs(zt).max(), 1e-9))
    r2 = run_bass_kernel_spmd(
        nc2, _in_maps(plan, 2, make_table(zt * s2), b2=b2, s2=s2),
        core_ids=list(range(NCORES)))
    y = decode_l2(plan, [np.asarray(r2.results[c]["out"], np.float32)
                         for c in range(NCORES)])

    last_run_info["exec_time_ns"] = [r1.exec_time_ns, r2.exec_time_ns]
    last_run_info["ncs"] = (nc1, nc2)
    last_run_info["plan"] = plan
    return y.astype(np.float32)
